# revision 22
# baseline (speedup 1.0000x reference)
"""Trainium2 Bass kernel for nn_EnetGnn (GNN message passing).

Reference computation (per batch n, with X = rgb_in[n] viewed as (C=1024, HW=1024),
nodes = columns of X):
  S[i,j]   = x_i . x_j                       (node similarity)
  nb(i)    = 16 smallest entries of S[i,:]   (k-NN, torch topk largest=False)
  M[m,:]   = relu(relu(X0_node_m @ w1 + b1) @ w2 + b2)   (MLP table; the
             reference gathers from the *globally flattened* node table, i.e.
             always batch 0's nodes)
  g_i      = mean_{m in nb(i)} M[m,:]
  A[i,j]   = g_i . g_j ; softmax over axis i (columns normalized)
  out      = X @ A_softmax + X

Implementation (8 cores, SPMD, one compiled program): core c handles batch
n = c//2 and channel-half h = c%2, with node columns rolled so the core's own
512 columns come first.  v3 design:
  - All big matmuls except OUT run in fp8(e4m3) DoubleRow perf mode (2
    contraction blocks per instruction): S = X^T X, H = w1^T X0, M = H^T w2,
    G^T = M^T P^T, A = G G^T.  Host pre-quantizes x/x0/w1/w2 to fp8
    pair-layout tiles.  Precision validated numerically (rel ~4e-3 vs 2e-2
    budget: fp8 S only affects top-16 *selection*; the MLP/G chain is
    non-negative so quantization noise averages out in the long sums).
  - top-16 threshold per row: max8 -> match_replace8 -> max8 (f32 out, so
    element 7 is directly the 16th-largest threshold), then the mask is a
    tensor_scalar is_ge on GpSimd (idle engine; DVE and ACT are both
    saturated in this phase).  Ties give a few rows >16 neighbors; impact
    measured small.
  - Engine budget in the topk phase (the pacing phase): DVE runs only the
    3-op chains (~35us), ACT only psum evacuations (sneg negate-copies, P^T
    copies, M relu) (~35us), PE runs S + MLP + transposes + first G^T half
    (keeps the HAM p-state warm), GpSimd the masks.
  - OUT = X E stays bf16 (E = exp overflows fp8); X^T comes pre-transposed
    from the host.  Softmax normalization via DVE divide by the
    partition-broadcast column sums (no Ln/Exp -> no ACT table thrash).
"""

import numpy as np
import ml_dtypes
from contextlib import ExitStack

from concourse import mybir, bacc, tile
from concourse.bass import ts
from concourse.bass_utils import run_bass_kernel_spmd
from concourse.masks import make_identity

F32 = mybir.dt.float32
BF16 = mybir.dt.bfloat16
FP8 = mybir.dt.float8e4
E4M3 = ml_dtypes.float8_e4m3
NPBF16 = ml_dtypes.bfloat16
P = 128
HWDIM = 1024   # number of nodes per batch (H*W)
CDIM = 1024    # channels
FDIM = 256     # MLP hidden dim
NB = 4         # batch
NCORES = 8
JH = HWDIM // 2  # nodes owned per core (columns rolled to front)
MINVAL = -1.0e30
DR = mybir.MatmulPerfMode.DoubleRow

Copy = mybir.ActivationFunctionType.Copy
Relu = mybir.ActivationFunctionType.Relu
Exp = mybir.ActivationFunctionType.Exp
Ln = mybir.ActivationFunctionType.Ln


def _build_program(nc: bacc.Bacc, use_b2: bool):
    x8d = nc.dram_tensor("x8", [4, P, 2, HWDIM], FP8, kind="ExternalInput").ap()
    x08d = nc.dram_tensor("x08", [4, P, 2, HWDIM], FP8, kind="ExternalInput").ap()
    w18d = nc.dram_tensor("w18", [4, P, 2, FDIM], FP8, kind="ExternalInput").ap()
    w28d = nc.dram_tensor("w28", [P, 2, CDIM], FP8, kind="ExternalInput").ap()
    xtd = nc.dram_tensor("xt", [8, P, CDIM], BF16, kind="ExternalInput").ap()
    xjd = nc.dram_tensor("xj", [8, P, JH], BF16, kind="ExternalInput").ap()
    b1d = nc.dram_tensor("b1", [2, P, 1], F32, kind="ExternalInput").ap()
    b2d = nc.dram_tensor("b2", [1, CDIM], F32, kind="ExternalInput").ap()
    outd = nc.dram_tensor("out", [CDIM, JH], F32, kind="ExternalOutput").ap()

    with tile.TileContext(nc) as tc, ExitStack() as ctx:
        persist = ctx.enter_context(tc.tile_pool(name="persist", bufs=1))

        # ---- constants ----
        id_b = persist.tile([P, P], BF16, tag="id_b", name="id_b")
        make_identity(nc, id_b[:])
        ones_row = persist.tile([1, P], F32, tag="ones_row", name="ones_row")
        nc.vector.memset(ones_row[:], 1.0)
        ones_col_b = persist.tile([P, 1], BF16, tag="ones_col_b", name="ones_col_b")
        nc.vector.memset(ones_col_b[:], 1.0)
        warm = persist.tile([1, 8], F32, tag="warm", name="warm")
        # hoist the one ACT function-table load into the DMA wait
        nc.scalar.activation(warm[0:1, :], ones_row[0:1, 0:8], Copy)

        # ---- persistent sbuf buffers ----
        x8 = [persist.tile([P, 2, HWDIM], FP8, tag=f"x8{q}", name=f"x8{q}")
              for q in range(4)]
        x08 = [persist.tile([P, 2, HWDIM], FP8, tag=f"x08{q}", name=f"x08{q}")
               for q in range(4)]
        w18 = [persist.tile([P, 2, FDIM], FP8, tag=f"w18{q}", name=f"w18{q}")
               for q in range(4)]
        w28 = persist.tile([P, 2, CDIM], FP8, tag="w28", name="w28")
        xt_sb = [persist.tile([P, CDIM], BF16, tag=f"xt{i}", name=f"xt{i}")
                 for i in range(8)]
        xj_sb = [persist.tile([P, JH], BF16, tag=f"xj{i}", name=f"xj{i}")
                 for i in range(8)]
        b1t = [persist.tile([P, 1], F32, tag=f"b1t{i}", name=f"b1t{i}")
               for i in range(2)]
        b2row = persist.tile([1, CDIM], F32, tag="b2row", name="b2row")
        h18 = persist.tile([P, 2, HWDIM], FP8, tag="h18", name="h18")
        m8 = [persist.tile([P, 2, CDIM], FP8, tag=f"m8{q}", name=f"m8{q}")
              for q in range(4)]
        # P^T, fp8: [j-local 128, j-block 8, m 1024]
        pt_sb = persist.tile([P, 8, HWDIM], FP8, tag="pt", name="pt")
        # G^T, fp8 pair layout over c: [c-local, slot, m], value = (sum_nb M)/4
        gt8 = [persist.tile([P, 2, HWDIM], FP8, tag=f"gt{q}", name=f"gt{q}")
               for q in range(4)]
        e_sb = [persist.tile([P, JH], BF16, tag=f"e{i}", name=f"e{i}")
                for i in range(8)]
        invbc = persist.tile([P, JH], F32, tag="invbc", name="invbc")
        inv_row = persist.tile([1, JH], F32, tag="inv_row", name="inv_row")

        # ---- early input DMAs (x8 first: S starts when these land) ----
        for q in range(4):
            nc.sync.dma_start(x8[q][:], x8d[q])
        for q in range(4):
            nc.sync.dma_start(w18[q][:], w18d[q])
        for i in range(2):
            nc.sync.dma_start(b1t[i][:], b1d[i])
        for q in range(4):
            nc.sync.dma_start(x08[q][:], x08d[q])

        with ExitStack() as s1:
            topk_pool = s1.enter_context(tc.tile_pool(name="topk", bufs=3))
            pm_pool = s1.enter_context(tc.tile_pool(name="pm", bufs=3))
            sneg = [None] * 8
            pmask = [None] * 8
            m8b = [None] * 8

            with ExitStack() as ps1:
                # ps_s bufs=1: S(t+1) waits sneg(t)'s evacuation, still under
                # the ~5us DVE topk pace; frees 2 banks so the A pools fit
                # alongside pt/g in the interleaved tail (2+2+2+1+1 = 8)
                ps_s = ps1.enter_context(
                    tc.tile_pool(name="ps_s", bufs=1, space="PSUM"))
                ps_pt = ps1.enter_context(
                    tc.tile_pool(name="ps_pt", bufs=2, space="PSUM"))
                ps_hm_scope = ExitStack()
                ps_hm = ps_hm_scope.enter_context(
                    tc.tile_pool(name="ps_hm", bufs=1, space="PSUM"))
                ps_g_holder = [None]

                # ---- interleaved chunk generators ----
                def h_chunk(ft):
                    # H[f in ft, :] = sum_c w1[c, f] x0[c, :], fp8 DoubleRow
                    ps = ps_hm.tile([P, HWDIM], F32, tag="HM", name="hps")
                    for ih in range(2):
                        for q in range(4):
                            nc.tensor.matmul(
                                ps[:, ts(ih, 512)], w18[q][:, :, ts(ft, P)],
                                x08[q][:, :, ts(ih, 512)],
                                start=(q == 0), stop=(q == 3), perf_mode=DR)
                    nc.scalar.activation(
                        h18[:, ft, :], ps[:], Relu, bias=b1t[ft][:])

                def m_chunk(mt):
                    # M[j in mt, :] = sum_f h1[f, j] w2[f, :], fp8 DR (1 pair)
                    ps = ps_hm.tile([P, HWDIM], F32, tag="HM", name="mps")
                    for ch in range(2):
                        nc.tensor.matmul(ps[:, ts(ch, 512)],
                                         h18[:, :, ts(mt, P)],
                                         w28[:, :, ts(ch, 512)],
                                         start=True, stop=not use_b2,
                                         perf_mode=DR)
                        if use_b2:
                            nc.tensor.matmul(ps[:, ts(ch, 512)], ones_row[:],
                                             b2row[0:1, ts(ch, 512)],
                                             start=False, stop=True,
                                             skip_group_check=True)
                    nc.scalar.activation(m8[mt // 2][:, mt % 2, :], ps[:], Relu)

                def close_hm():
                    ps_hm_scope.close()
                    ps_g_holder[0] = ps1.enter_context(
                        tc.tile_pool(name="ps_g", bufs=2, space="PSUM"))

                def g_chunk(k):
                    # G^T[c in cb, m-half mh] = sum_j M[j, c] P^T[j, m]
                    cb, mh = k % 8, k // 8
                    ps = ps_g_holder[0].tile([P, 512], F32, tag="G", name="gps")
                    for q in range(4):
                        nc.tensor.matmul(
                            ps[:], m8[q][:, :, ts(cb, P)],
                            pt_sb[:, 2 * q:2 * q + 2, mh * 512:mh * 512 + 512],
                            start=(q == 0), stop=(q == 3), perf_mode=DR)
                    dst = gt8[cb // 2][:, cb % 2, ts(mh, 512)]
                    if k % 2 == 0:
                        nc.scalar.activation(dst, ps[:], Copy, scale=0.25)
                    else:
                        nc.vector.tensor_scalar(
                            out=dst, in0=ps[:], scalar1=0.25, scalar2=None,
                            op0=mybir.AluOpType.mult)

                H, M, G = h_chunk, m_chunk, g_chunk
                chunks = (
                    [lambda: H(0), lambda: H(1)]
                    + [lambda k=k: M(k) for k in range(8)]
                    + [lambda: close_hm()]
                    + [lambda k=k: G(k) for k in range(8)]   # m-half 0
                )
                sched = [0, 1, 2, 2, 2, 3, 3, 3]  # 16 of 19; rest post-loop

                def topk_head(t):
                    # DVE chain for tile t: top-8, zap, next-8 (f32 out so
                    # [:, 7] is directly the is_ge threshold)
                    m8a = topk_pool.tile([P, 8], BF16, tag="m8a", name="m8a")
                    szap = topk_pool.tile([P, HWDIM], BF16, tag="szap",
                                          name="szap", bufs=2)
                    m8b[t] = topk_pool.tile([P, 8], F32, tag="m8b", name="m8b")
                    nc.vector.max(out=m8a[:], in_=sneg[t][:])
                    nc.vector.match_replace(
                        out=szap[:], in_to_replace=m8a[:], in_values=sneg[t][:],
                        imm_value=MINVAL)
                    nc.vector.max(out=m8b[t][:], in_=szap[:])

                def mask_tail(t):
                    # mask = sneg >= 16th largest (DVE 4x mode on bf16;
                    # gpsimd tensor_scalar wedges the device: NRT 101)
                    pmask[t] = pm_pool.tile([P, HWDIM], BF16, tag="pm",
                                            name="pm")
                    nc.vector.tensor_scalar(
                        out=pmask[t][:], in0=sneg[t][:],
                        scalar1=m8b[t][:, 7:8], scalar2=None,
                        op0=mybir.AluOpType.is_ge)

                def pt_transpose(t):
                    # pmask[t] (i-rows in t, all j) -> P^T columns m in t
                    ps = ps_pt.tile([P, 8, P], BF16, tag="PT", name="ptps")
                    for jb in range(8):
                        nc.tensor.transpose(
                            ps[:, jb], pmask[t][:, ts(jb, P)], id_b[:])
                    nc.scalar.activation(
                        pt_sb[:, :, ts(t, P)], ps[:], Copy)

                for t in range(8):
                    # S tile t: psum[:, jh-half] = sum over 4 c-pairs, fp8 DR
                    ps = ps_s.tile([P, HWDIM], F32, tag="S")
                    for jh in range(2):
                        for q in range(4):
                            nc.tensor.matmul(
                                ps[:, ts(jh, 512)], x8[q][:, :, ts(t, P)],
                                x8[q][:, :, ts(jh, 512)],
                                start=(q == 0), stop=(q == 3), perf_mode=DR)
                    sneg[t] = topk_pool.tile([P, HWDIM], BF16, tag="sneg",
                                             name="sneg", bufs=3)
                    nc.scalar.activation(sneg[t][:], ps[:], Copy, scale=-1.0)
                    if t >= 1:
                        mask_tail(t - 1)
                        pt_transpose(t - 1)
                    topk_head(t)
                    if t == 0:
                        nc.sync.dma_start(w28[:], w28d[:, :, :])
                    if t == 2:
                        for i in range(8):
                            nc.sync.dma_start(xt_sb[i][:], xtd[i])
                        for i in range(8):
                            nc.sync.dma_start(xj_sb[i][:], xjd[i])
                        nc.sync.dma_start(b2row[:], b2d[:, :])
                    for _ in range(sched[t]):
                        chunks.pop(0)()
                mask_tail(7)
                pt_transpose(7)
                for chunk in chunks:
                    chunk()
                # ---- G^T m-half 1 interleaved with A tiles mt 0-3 (which
                # need only the mh=0 half of G^T: both their lhsT slice and
                # the own-j rhs are m-cols 0-511) — fills the PE dependency
                # gaps between G chunks and keeps the HAM p-state warm ----
                ps_a = ps1.enter_context(tc.tile_pool(name="ps_a", bufs=1,
                                                      space="PSUM"))
                ps_cs = ps1.enter_context(tc.tile_pool(name="ps_cs", bufs=1,
                                                       space="PSUM"))
                cs = ps_cs.tile([1, JH], F32, tag="CS")

                def a_chunk(mt):
                    ps = ps_a.tile([P, JH], F32, tag="A")
                    for q in range(4):
                        nc.tensor.matmul(
                            ps[:], gt8[q][:, :, ts(mt, P)], gt8[q][:, :, 0:JH],
                            start=(q == 0), stop=(q == 3), perf_mode=DR)
                    # psum = (1/16) sum_c (sum_nb M)_i (sum_nb M)_j; true A
                    # is the mean over 16 neighbors each side -> exp(psum/16)
                    nc.scalar.activation(e_sb[mt][:], ps[:], Exp,
                                         scale=1.0 / 16.0)
                    nc.tensor.matmul(
                        cs[0:1, :], ones_col_b[:], e_sb[mt][:],
                        start=(mt == 0), stop=(mt == 7))

                # A mt 0-3 ride between the first G-h1 chunks; A mt 4-7 need
                # every G-h1 chunk (their lhsT spans all cb at m-cols 512+),
                # so they must follow g15 in the in-order PE queue
                for step in (lambda: g_chunk(8), lambda: a_chunk(0),
                             lambda: g_chunk(9), lambda: a_chunk(1),
                             lambda: g_chunk(10), lambda: a_chunk(2),
                             lambda: g_chunk(11), lambda: a_chunk(3),
                             lambda: g_chunk(12), lambda: g_chunk(13),
                             lambda: g_chunk(14), lambda: g_chunk(15),
                             lambda: a_chunk(4), lambda: a_chunk(5),
                             lambda: a_chunk(6), lambda: a_chunk(7)):
                    step()
                # 1/colsum = exp(-ln(colsum)) on ACT (DVE reciprocal on one
                # partition costs ~6.5us; ACT Reciprocal is blocked in bass)
                nc.scalar.activation(inv_row[0:1, :], cs[0:1, :], Ln)
                nc.scalar.activation(inv_row[0:1, :], inv_row[0:1, :], Exp,
                                     scale=-1.0)
                nc.gpsimd.partition_broadcast(invbc[:], inv_row[0:1, :],
                                              channels=P)

        # ---- OUT = X E (bf16), scale by 1/colsum, add identity ----
        with ExitStack() as s5:
            ps_o = s5.enter_context(tc.tile_pool(name="ps_o", bufs=4,
                                                 space="PSUM"))
            fin_pool = s5.enter_context(tc.tile_pool(name="fin", bufs=4))
            for cb in range(8):
                ps = ps_o.tile([P, JH], F32, tag="O")
                for mt in range(8):
                    nc.tensor.matmul(
                        ps[:], xt_sb[mt][:, ts(cb, P)], e_sb[mt][:],
                        start=(mt == 0), stop=(mt == 7))
                tmp = fin_pool.tile([P, JH], F32, tag="tmp", name="tmp")
                nc.vector.tensor_tensor(
                    out=tmp[:], in0=ps[:], in1=invbc[:],
                    op=mybir.AluOpType.mult)
                outt = fin_pool.tile([P, JH], F32, tag="outt", name="outt")
                nc.vector.tensor_tensor(
                    out=outt[:], in0=tmp[:], in1=xj_sb[cb][:],
                    op=mybir.AluOpType.add)
                nc.sync.dma_start(outd[ts(cb, P), :], outt[:])

    return nc


_NC = {}


def _get_nc(use_b2=False):
    if use_b2 not in _NC:
        nc = bacc.Bacc("TRN2", target_bir_lowering=False, debug=False,
                       num_devices=NCORES)
        _build_program(nc, use_b2)
        nc.compile()
        _NC[use_b2] = nc
    return _NC[use_b2]


def _fp8_pairs(a):
    """(1024, F) f32 -> [4, 128, 2, F] e4m3 pair-layout tiles."""
    a8 = np.clip(a, -240.0, 240.0).astype(E4M3)
    return np.ascontiguousarray(
        a8.reshape(4, 2, P, -1).transpose(0, 2, 1, 3))


def _in_maps(cat, rgb_in, w1, b1, w2, b2):
    del cat  # unused by the reference computation
    x4 = np.ascontiguousarray(rgb_in.reshape(NB, CDIM, HWDIM)).astype(np.float32)
    w1 = np.ascontiguousarray(w1, dtype=np.float32)
    w2 = np.ascontiguousarray(w2, dtype=np.float32)
    b1r = np.ascontiguousarray(b1.reshape(2, P, 1), dtype=np.float32)
    b2r = np.ascontiguousarray(b2.reshape(1, CDIM), dtype=np.float32)
    w18 = _fp8_pairs(w1)
    w28 = np.ascontiguousarray(
        np.clip(w2, -240, 240).astype(E4M3).reshape(2, P, CDIM)
        .transpose(1, 0, 2))
    maps = []
    for core in range(NCORES):
        n, qh = core // 2, core % 2
        roll = (lambda a: a) if qh == 0 else (
            lambda a: np.ascontiguousarray(np.concatenate(
                [a[:, JH:], a[:, :JH]], axis=1)))
        xr = roll(x4[n])
        x0r = roll(x4[0])
        maps.append({
            "x8": _fp8_pairs(xr),
            "x08": _fp8_pairs(x0r),
            "w18": w18,
            "w28": w28,
            "xt": np.ascontiguousarray(
                xr.T.astype(NPBF16).reshape(8, P, CDIM)),
            "xj": np.ascontiguousarray(
                xr[:, :JH].astype(NPBF16).reshape(8, P, JH)),
            "b1": b1r,
            "b2": b2r,
        })
    return maps


def _assemble(results, rgb_shape):
    N, C, H, W = rgb_shape
    out = np.empty((N, C, H * W), np.float32)
    for core, res in enumerate(results):
        n, q = core // 2, core % 2
        out[n, :, q * JH:(q + 1) * JH] = res["out"]
    return out.reshape(N, C, H, W)


def run_on_hw(cat, rgb_in, w1, b1, w2, b2, trace=False, **kw):
    nc = _get_nc(use_b2=bool(np.any(np.asarray(b2))))
    maps = _in_maps(cat, rgb_in, w1, b1, w2, b2)
    res = run_bass_kernel_spmd(nc, maps, core_ids=list(range(NCORES)),
                               trace=trace, **kw)
    out = _assemble(res.results, rgb_in.shape)
    return out, res


def kernel(cat, rgb_in, w1, b1, w2, b2, gnn_iterations=1, k=16):
    assert int(gnn_iterations) == 1 and int(k) == 16
    cat = np.asarray(cat)
    rgb_in = np.asarray(rgb_in, dtype=np.float32)
    out, _ = run_on_hw(cat, rgb_in, np.asarray(w1), np.asarray(b1),
                       np.asarray(w2), np.asarray(b2))
    return out
